# revision 2
# baseline (speedup 1.0000x reference)
"""LightGCN 3-layer message passing on 8 TRN2 NeuronCores (Bass, raw bacc).

Design (dst-sharded, uniform SPMD schedule):
- Node ids are permuted (user/item 2:1 interleave) so degree is balanced
  across the 8 contiguous dst shards of R=18816 rows each.
- Per layer, per 128-row dst window, per 32768-row src chunk (int16
  dma_gather limit), edges are packed into 128-slot blocks. The dma_gather
  ucode instruction (<=512 idx/call) pulls h[src] f32 rows (256B) from the
  replicated DRAM table into SBUF; DVE multiplies by edge weight
  (stride-0 broadcast) into bf16 M tiles; the TensorEngine multiplies
  one-hot S01 matrices (built on DVE per block from local-dst via
  is_equal against an iota tile) accumulating each window's blocks in a
  PSUM bank; DVE flushes psum into the SBUF h accumulator per window.
- Between layers the shard h is exported to DRAM and an AllGather
  (gpsimd) rebuilds the replicated [NTOT, 64] f32 gather table.
- acc = x + h1 + h2 + h3 stays in SBUF f32 and is exported at the end;
  the host divides by 4 and slices the queried user/item rows.

The instruction schedule is identical on all 8 cores (block counts maxed
across cores); only the per-core data images differ. All semaphore
thresholds are static; waits are emitted monotonically per engine.

Falls back to an exact numpy implementation if anything in the device
path fails.
"""

import sys
from dataclasses import dataclass, field

import numpy as np

N_USERS = 100_000
N_ITEMS = 50_000
N_NODES = N_USERS + N_ITEMS
D = 64
NUM_LAYERS = 3
NCORE = 8
WIN = 128
R = 18816                 # rows per core, multiple of 128
NTOT = NCORE * R          # 150528 padded node rows
CH = 32768                # gather chunk rows (int16 index limit)
MAXB = 4                  # blocks per gather call (512 idx)
SGRP = 4                  # blocks per S megatile

LAST_EXEC_NS = None       # TimelineSim estimate of the NEFF, set by kernel()

_CACHE = {}


# --------------------------------------------------------------------------
# host-side schedule
# --------------------------------------------------------------------------

@dataclass
class Schedule:
    R: int
    CH: int
    NW: int
    NCH: int
    B: np.ndarray
    TB: int
    calls: list = field(default_factory=list)
    win_first_gb: dict = field(default_factory=dict)
    win_last_gb: dict = field(default_factory=dict)
    nonempty_windows: list = field(default_factory=list)
    idx_img: list = field(default_factory=list)
    w_img: list = field(default_factory=list)
    ldst_img: list = field(default_factory=list)


def build_schedule(src_g, dst_g, wts, ntot, R, CH):
    NW = R // WIN
    NCH = -(-ntot // CH)
    core = dst_g // R
    lrow = dst_g - core * R
    w = lrow // WIN
    ldst = lrow % WIN
    ch = src_g // CH
    lsrc = (src_g - ch * CH).astype(np.int16)

    gkey = (core * NW + w) * NCH + ch
    order = np.argsort(gkey, kind="stable")
    E = src_g.shape[0]
    cnt = np.bincount(gkey, minlength=NCORE * NW * NCH).reshape(NCORE, NW, NCH)
    B = -(-cnt.max(axis=0) // 128)
    TB = int(B.sum())

    bb = np.zeros(NW * NCH + 1, np.int64)
    np.cumsum(B.reshape(-1), out=bb[1:])
    block_base = bb[:-1].reshape(NW, NCH)

    sk = gkey[order]
    grp_start = np.r_[0, np.flatnonzero(np.diff(sk)) + 1]
    grp_len = np.diff(np.r_[grp_start, E])
    pos = np.arange(E) - np.repeat(grp_start, grp_len)

    slot = np.empty(E, np.int64)
    slot[order] = block_base[w[order], ch[order]] * 128 + pos

    total_slots = TB * 128
    sched = Schedule(R=R, CH=CH, NW=NW, NCH=NCH, B=B, TB=TB)

    for c in range(NCORE):
        m = core == c
        idx16 = np.zeros(total_slots, np.int16)
        wv = np.zeros(total_slots, np.float32)
        lv = np.zeros(total_slots, np.float32)
        idx16[slot[m]] = lsrc[m]
        wv[slot[m]] = wts[m]
        lv[slot[m]] = ldst[m].astype(np.float32)
        img16 = np.ascontiguousarray(idx16.reshape(-1, 16).T)
        sched.idx_img.append(np.ascontiguousarray(np.tile(img16, (8, 1))))
        sched.w_img.append(np.ascontiguousarray(wv.reshape(TB, 128).T))
        sched.ldst_img.append(np.ascontiguousarray(lv.reshape(TB, 128).T))

    for wi in range(NW):
        row_blocks = 0
        for chi in range(NCH):
            b = int(B[wi, chi])
            if b == 0:
                continue
            gb0 = int(block_base[wi, chi])
            if row_blocks == 0:
                sched.win_first_gb[wi] = gb0
            sched.win_last_gb[wi] = gb0 + b - 1
            row_blocks += b
            off = 0
            while off < b:
                nb = min(MAXB, b - off)
                sched.calls.append((wi, chi, gb0 + off, nb))
                off += nb
        if row_blocks > 0:
            sched.nonempty_windows.append(wi)
    return sched


# --------------------------------------------------------------------------
# device program
# --------------------------------------------------------------------------

def build_program(s: Schedule, ntot: int, nlayers: int = 3):
    import concourse.bacc as bacc
    import concourse.bass as bass
    import concourse.mybir as mybir
    from concourse._compat import get_trn_type
    from concourse.library_config import mlp
    from contextlib import ExitStack

    R, CH, NW, NCH, TB = s.R, s.CH, s.NW, s.NCH, s.TB
    NG = -(-TB // SGRP)
    C = len(s.calls)

    nc = bacc.Bacc(get_trn_type() or "TRN2", debug=True)

    xshard = nc.dram_tensor("xshard", [R, D], mybir.dt.float32, kind="ExternalInput")
    idx_in = nc.dram_tensor("idx_img", [128, 8 * TB], mybir.dt.int16, kind="ExternalInput")
    w_in = nc.dram_tensor("w_img", [128, TB], mybir.dt.float32, kind="ExternalInput")
    ldst_in = nc.dram_tensor("ldst_img", [128, TB], mybir.dt.float32, kind="ExternalInput")
    iota_in = nc.dram_tensor("iota", [128, 128], mybir.dt.float32, kind="ExternalInput")
    acc_out = nc.dram_tensor("acc_out", [R, D], mybir.dt.float32, kind="ExternalOutput")

    t_a = nc.dram_tensor("t_a", [ntot, D], mybir.dt.float32, addr_space="Shared")
    t_b = nc.dram_tensor("t_b", [ntot, D], mybir.dt.float32, addr_space="Shared")
    myshard = nc.dram_tensor("myshard", [R, D], mybir.dt.float32)

    tables = [t_a, t_b, t_a]

    def mm_thr(layer, gb):
        return layer * TB + gb + 1

    flush_index = {}
    for li in range(nlayers):
        for k, wi in enumerate(s.nonempty_windows):
            flush_index[(li, wi)] = li * len(s.nonempty_windows) + k + 1
    NF = len(s.nonempty_windows)

    def group_last_gb(g):
        return min((g + 1) * SGRP, TB) - 1

    def shard_ap(t):
        return bass.AP(t, 0, [[D, 128], [WIN * D, NW], [1, D]])

    class W:
        def __init__(self, eng):
            self.eng = eng
            self.best = {}

        def __call__(self, sem, val):
            if val <= 0:
                return
            if self.best.get(id(sem), -1) >= val:
                return
            self.best[id(sem)] = val
            self.eng.wait_ge(sem, val)

    with ExitStack() as st:
        block = st.enter_context(nc.Block())
        sb = lambda name, shape, dt: st.enter_context(nc.sbuf_tensor(name, shape, dt))
        sem = lambda name: st.enter_context(nc.semaphore(name))

        idx_t = sb("idx_t", [128, 8 * TB], mybir.dt.int16)
        w_t = sb("w_t", [128, TB], mybir.dt.float32)
        ldst_t = sb("ldst_t", [128, TB], mybir.dt.float32)
        iota_t = sb("iota_t", [128, 128], mybir.dt.float32)
        acc_t = sb("acc_t", [128, NW, D], mybir.dt.float32)
        h_t = sb("h_t", [128, NW, D], mybir.dt.float32)
        G = [sb(f"G{i}", [128, MAXB, D], mybir.dt.float32) for i in range(4)]
        M = [sb(f"M{i}", [128, MAXB, D], mybir.dt.bfloat16) for i in range(4)]
        S = [sb(f"S{i}", [128, SGRP, 128], mybir.dt.bfloat16) for i in range(2)]
        ps = [st.enter_context(nc.psum_tensor(f"ps{i}", [128, 512], mybir.dt.float32))
              for i in range(8)]

        ld = sem("ld")
        g_sem = sem("g_sem")
        m_done = sem("m_done")
        sb_sem = sem("sb_sem")
        mm_sem = sem("mm_sem")
        f_sem = sem("f_sem")
        cc_sem = sem("cc_sem")
        e_sem = sem("e_sem")
        aa_sem = sem("aa_sem")
        o_sem = sem("o_sem")

        @block.sync
        def _(s_eng):
            wt = W(s_eng)
            s_eng.dma_start(idx_t[:], idx_in[:]).then_inc(ld, 16)
            s_eng.dma_start(w_t[:], w_in[:]).then_inc(ld, 16)
            s_eng.dma_start(ldst_t[:], ldst_in[:]).then_inc(ld, 16)
            s_eng.dma_start(iota_t[:], iota_in[:]).then_inc(ld, 16)
            s_eng.dma_start(acc_t[:], shard_ap(xshard)).then_inc(ld, 16)
            s_eng.dma_start(myshard[:], xshard[:]).then_inc(ld, 16)
            for li in range(nlayers - 1):
                wt(f_sem, (li + 1) * NF)
                wt(cc_sem, li + 1)
                s_eng.dma_start(shard_ap(myshard), h_t[:]).then_inc(e_sem, 16)
            wt(aa_sem, nlayers)
            s_eng.dma_start(shard_ap(acc_out), acc_t[:]).then_inc(o_sem, 16)
            wt(o_sem, 16)

        @block.gpsimd
        def _(g_eng):
            wt = W(g_eng)
            g_eng.load_library(mlp)
            wt(ld, 16 * 6)
            g_eng.collective_compute(
                "AllGather", mybir.AluOpType.bypass,
                replica_groups=[list(range(NCORE))],
                ins=[myshard[:]], outs=[t_a[:]],
            ).then_inc(cc_sem)
            for li in range(nlayers):
                wt(cc_sem, li + 1)
                tbl = tables[li]
                for q, (wi, chi, gb0, nb) in enumerate(s.calls):
                    Q = li * C + q
                    wt(m_done, Q - 3)
                    ni = 128 * nb
                    lo = chi * CH
                    hi = min(lo + CH, ntot)
                    g_eng.dma_gather(
                        G[Q % 4][:, :nb, :], tbl[lo:hi],
                        idx_t[:, 8 * gb0: 8 * (gb0 + nb)],
                        ni, ni, D,
                    ).then_inc(g_sem, 16)
                if li < nlayers - 1:
                    wt(e_sem, 16 * (li + 1))
                    g_eng.collective_compute(
                        "AllGather", mybir.AluOpType.bypass,
                        replica_groups=[list(range(NCORE))],
                        ins=[myshard[:]], outs=[tables[li + 1][:]],
                    ).then_inc(cc_sem)

        @block.vector
        def _(v):
            wt = W(v)
            wt(ld, 16 * 6)
            v.memset(h_t[:], 0.0)
            for li in range(nlayers):
                wt(e_sem, 16 * li)
                for q, (wi, chi, gb0, nb) in enumerate(s.calls):
                    Q = li * C + q
                    for j in range(nb):
                        b = gb0 + j
                        g = b // SGRP
                        gg = li * NG + g
                        if gg - 2 >= 0:
                            pl, pg = divmod(gg - 2, NG)
                            wt(mm_sem, mm_thr(pl, group_last_gb(pg)))
                        v.tensor_tensor(
                            out=S[g % 2][:, b - g * SGRP, :],
                            in0=ldst_t[:, b:b + 1].to_broadcast([128, 128]),
                            in1=iota_t[:],
                            op=mybir.AluOpType.is_equal,
                        ).then_inc(sb_sem)
                    wt(g_sem, 16 * (Q + 1))
                    if Q - 4 >= 0:
                        pl, pqq = divmod(Q - 4, C)
                        pgb0, pnb = s.calls[pqq][2], s.calls[pqq][3]
                        wt(mm_sem, mm_thr(pl, pgb0 + pnb - 1))
                    v.tensor_tensor(
                        out=M[Q % 4][:, :nb, :],
                        in0=G[Q % 4][:, :nb, :],
                        in1=w_t[:, gb0:gb0 + nb].to_broadcast([128, nb, D]),
                        op=mybir.AluOpType.mult,
                    ).then_inc(m_done)
                    if gb0 + nb - 1 == s.win_last_gb.get(wi, -1):
                        wt(mm_sem, mm_thr(li, gb0 + nb - 1))
                        v.tensor_copy(
                            out=h_t[:, wi, :],
                            in_=ps[wi % 8][:, :D],
                        ).then_inc(f_sem)
                wt(f_sem, (li + 1) * NF)
                v.tensor_add(out=acc_t[:], in0=acc_t[:], in1=h_t[:]).then_inc(aa_sem)

        @block.tensor
        def _(t):
            wt = W(t)
            for li in range(nlayers):
                for q, (wi, chi, gb0, nb) in enumerate(s.calls):
                    Q = li * C + q
                    wt(m_done, Q + 1)
                    for j in range(nb):
                        b = gb0 + j
                        g = b // SGRP
                        wt(sb_sem, li * TB + b + 1)
                        first = b == s.win_first_gb.get(wi, -2)
                        last = b == s.win_last_gb.get(wi, -2)
                        if first:
                            fi = flush_index[(li, wi)]
                            wt(f_sem, fi - 8)
                        t.matmul(
                            ps[wi % 8][:, :D],
                            S[g % 2][:, b - g * SGRP, :],
                            M[Q % 4][:, j, :],
                            start=first, stop=last,
                        ).then_inc(mm_sem)

    nc.compile()
    return nc


# --------------------------------------------------------------------------
# entry points
# --------------------------------------------------------------------------

def _permutation():
    pi = np.empty(N_NODES, np.int64)
    u = np.arange(N_USERS)
    pi[:N_USERS] = 3 * (u // 2) + (u % 2)
    i = np.arange(N_ITEMS)
    pi[N_USERS:] = 3 * i + 2
    return pi


def _kernel_device(user_embedding, item_embedding, edge_weight, edge_index,
                   user_id, item_id):
    global LAST_EXEC_NS
    from concourse import bass_utils

    pi = _permutation()
    src = np.asarray(edge_index[0], np.int64)
    dst = np.asarray(edge_index[1], np.int64)
    w = np.asarray(edge_weight, np.float32)
    x = np.concatenate([
        np.asarray(user_embedding, np.float32),
        np.asarray(item_embedding, np.float32),
    ])
    xp = np.zeros((NTOT, D), np.float32)
    xp[pi] = x

    key = (src.shape[0],)
    if "sched_hash" in _CACHE and _CACHE.get("edges_key") == key:
        pass  # reuse compiled program only if the edge structure is identical
    sched = build_schedule(pi[src], pi[dst], w, NTOT, R, CH)
    shash = hash((sched.TB, tuple(sched.calls[:50]), len(sched.calls)))
    if _CACHE.get("sched_hash") != shash:
        _CACHE["nc"] = build_program(sched, NTOT, NUM_LAYERS)
        _CACHE["sched_hash"] = shash
        _CACHE["edges_key"] = key
        try:
            from concourse.timeline_sim import TimelineSim
            _CACHE["est_ns"] = int(TimelineSim(_CACHE["nc"]).simulate())
        except Exception:
            _CACHE["est_ns"] = None
    nc = _CACHE["nc"]
    LAST_EXEC_NS = _CACHE.get("est_ns")

    iota = np.tile(np.arange(128, dtype=np.float32), (128, 1))
    in_maps = []
    for c in range(NCORE):
        in_maps.append({
            "xshard": np.ascontiguousarray(xp[c * R:(c + 1) * R]),
            "idx_img": sched.idx_img[c],
            "w_img": sched.w_img[c],
            "ldst_img": sched.ldst_img[c],
            "iota": iota,
        })
    res = bass_utils.run_bass_kernel_spmd(nc, in_maps, list(range(NCORE)), trace=False)
    acc = np.concatenate([res.results[c]["acc_out"] for c in range(NCORE)], axis=0)

    final = acc[pi] * np.float32(1.0 / (NUM_LAYERS + 1))
    u_embed = np.ascontiguousarray(final[:N_USERS][np.asarray(user_id, np.int64)])
    i_embed = np.ascontiguousarray(final[N_USERS:][np.asarray(item_id, np.int64)])
    if not (np.isfinite(u_embed).all() and np.isfinite(i_embed).all()):
        raise FloatingPointError("non-finite device output")
    return (u_embed, i_embed)


def _kernel_numpy(user_embedding, item_embedding, edge_weight, edge_index,
                  user_id, item_id):
    """Exact host fallback (feature-major reduceat segment sums)."""
    xt = np.empty((D, N_NODES), np.float32)
    xt[:, :N_USERS] = np.asarray(user_embedding, np.float32).T
    xt[:, N_USERS:] = np.asarray(item_embedding, np.float32).T
    src = np.asarray(edge_index[0], np.int64)
    dst = np.asarray(edge_index[1], np.int64)
    w = np.asarray(edge_weight, np.float32)
    user_id = np.asarray(user_id, np.int64)
    item_id = np.asarray(item_id, np.int64)

    order = np.argsort(dst.astype(np.uint32), kind="stable")
    src_s, dst_s, w_s = src[order], dst[order], w[order]
    starts = np.flatnonzero(np.diff(dst_s, prepend=dst_s[0] - 1))
    rows = dst_s[starts]

    def seg(ht):
        m = np.take(ht, src_s, axis=1, mode="clip")
        np.multiply(m, w_s[None, :], out=m)
        out = np.zeros_like(ht)
        out[:, rows] = np.add.reduceat(m, starts, axis=1)
        return out

    h1 = seg(xt)
    h2 = seg(h1)
    h3 = seg(h2)
    qu = user_id
    qi = N_USERS + item_id
    fu = (xt[:, qu] + h1[:, qu] + h2[:, qu] + h3[:, qu]).T * np.float32(0.25)
    fi = (xt[:, qi] + h1[:, qi] + h2[:, qi] + h3[:, qi]).T * np.float32(0.25)
    return (np.ascontiguousarray(fu), np.ascontiguousarray(fi))


def kernel(user_embedding, item_embedding, edge_weight, edge_index,
           user_id, item_id):
    try:
        return _kernel_device(user_embedding, item_embedding, edge_weight,
                              edge_index, user_id, item_id)
    except Exception as e:
        print(f"kernel: device path failed ({type(e).__name__}: {e}); "
              f"using host fallback", file=sys.stderr)
        return _kernel_numpy(user_embedding, item_embedding, edge_weight,
                             edge_index, user_id, item_id)


# revision 7
# speedup vs baseline: 1.0194x; 1.0194x over previous
"""LightGCN 3-layer message passing on 8 TRN2 NeuronCores (Bass, raw bacc).

Design (dst-sharded, uniform SPMD schedule):
- Node ids are permuted (user/item 2:1 interleave) so degree is balanced
  across the 8 contiguous dst shards of R=18816 rows each.
- Per layer, per 128-row dst window, per 32768-row src chunk (int16
  dma_gather limit), edges are packed into 128-slot blocks. The dma_gather
  ucode instruction (<=512 idx/call) pulls h[src] f32 rows (256B) from the
  replicated DRAM table into SBUF; DVE multiplies by edge weight
  (stride-0 broadcast) into bf16 M tiles; the TensorEngine multiplies
  one-hot S01 matrices (built on DVE per block from local-dst via
  is_equal against an iota tile) accumulating each window's blocks in a
  PSUM bank; DVE flushes psum into the SBUF h accumulator per window.
- Between layers the shard h is exported to DRAM and an AllGather
  (gpsimd) rebuilds the replicated [NTOT, 64] f32 gather table.
- acc = x + h1 + h2 + h3 stays in SBUF f32 and is exported at the end;
  the host divides by 4 and slices the queried user/item rows.

The instruction schedule is identical on all 8 cores (block counts maxed
across cores); only the per-core data images differ. All semaphore
thresholds are static; waits are emitted monotonically per engine.

Falls back to an exact numpy implementation if anything in the device
path fails.
"""

import sys
from dataclasses import dataclass, field

import numpy as np

N_USERS = 100_000
N_ITEMS = 50_000
N_NODES = N_USERS + N_ITEMS
D = 64
NUM_LAYERS = 3
NCORE = 8
WIN = 128
R = 18816                 # rows per core, multiple of 128
NTOT = NCORE * R          # 150528 padded node rows
CH = 32768                # gather chunk rows (int16 index limit)
MAXB = 4                  # blocks per gather call (512 idx)
SGRP = 4                  # blocks per S megatile

LAST_EXEC_NS = None       # TimelineSim estimate of the NEFF, set by kernel()

_CACHE = {}


# --------------------------------------------------------------------------
# host-side schedule
# --------------------------------------------------------------------------

@dataclass
class Schedule:
    R: int
    CH: int
    NW: int
    NCH: int
    B: np.ndarray
    TB: int
    calls: list = field(default_factory=list)
    win_first_gb: dict = field(default_factory=dict)
    win_last_gb: dict = field(default_factory=dict)
    nonempty_windows: list = field(default_factory=list)
    idx_img: list = field(default_factory=list)
    w_img: list = field(default_factory=list)
    ldst_img: list = field(default_factory=list)


SWG = 8  # windows per super-window (psum bank count)


def build_schedule(src_g, dst_g, wts, ntot, R, CH):
    """Blocks ordered by (super-window, chunk, window); gather calls pack up
    to MAXB consecutive blocks within one (super-window, chunk) run, crossing
    window boundaries. PSUM bank lifetime stays within a super-window of
    SWG=8 windows (bank = window % 8).

    calls: (chi, gb0, nb, [window of each block])
    """
    NW = R // WIN
    NCH = -(-ntot // CH)
    core = dst_g // R
    lrow = dst_g - core * R
    w = lrow // WIN
    ldst = lrow % WIN
    ch = src_g // CH
    lsrc = (src_g - ch * CH).astype(np.int16)

    gkey = (core * NW + w) * NCH + ch
    order = np.argsort(gkey, kind="stable")
    E = src_g.shape[0]
    cnt = np.bincount(gkey, minlength=NCORE * NW * NCH).reshape(NCORE, NW, NCH)
    B = -(-cnt.max(axis=0) // 128)
    TB = int(B.sum())

    # block-group order: (sw, ch, w)
    wc_order = sorted(
        ((wi, chi) for wi in range(NW) for chi in range(NCH)),
        key=lambda t: (t[0] // SWG, t[1], t[0]),
    )
    block_base = np.zeros((NW, NCH), np.int64)
    nb_cum = 0
    for (wi, chi) in wc_order:
        block_base[wi, chi] = nb_cum
        nb_cum += int(B[wi, chi])
    assert nb_cum == TB

    sk = gkey[order]
    grp_start = np.r_[0, np.flatnonzero(np.diff(sk)) + 1]
    grp_len = np.diff(np.r_[grp_start, E])
    pos = np.arange(E) - np.repeat(grp_start, grp_len)

    slot = np.empty(E, np.int64)
    slot[order] = block_base[w[order], ch[order]] * 128 + pos

    total_slots = TB * 128
    sched = Schedule(R=R, CH=CH, NW=NW, NCH=NCH, B=B, TB=TB)

    for c in range(NCORE):
        m = core == c
        idx16 = np.zeros(total_slots, np.int16)
        wv = np.zeros(total_slots, np.float32)
        lv = np.zeros(total_slots, np.float32)
        idx16[slot[m]] = lsrc[m]
        wv[slot[m]] = wts[m]
        lv[slot[m]] = ldst[m].astype(np.float32)
        img16 = np.ascontiguousarray(idx16.reshape(-1, 16).T)
        sched.idx_img.append(np.ascontiguousarray(np.tile(img16, (8, 1))))
        sched.w_img.append(np.ascontiguousarray(wv.reshape(TB, 128).T))
        sched.ldst_img.append(np.ascontiguousarray(lv.reshape(TB, 128).T))

    # window first/last block + calls packed within (sw, ch) runs
    win_of_block = np.empty(TB, np.int64)
    for (wi, chi) in wc_order:
        b = int(B[wi, chi])
        if b == 0:
            continue
        gb0 = int(block_base[wi, chi])
        win_of_block[gb0:gb0 + b] = wi
        if wi not in sched.win_first_gb:
            sched.win_first_gb[wi] = gb0
        sched.win_first_gb[wi] = min(sched.win_first_gb[wi], gb0)
        sched.win_last_gb[wi] = max(sched.win_last_gb.get(wi, -1), gb0 + b - 1)

    # runs of consecutive blocks sharing (sw, ch)
    run_start = 0
    prev = None
    runs = []
    for (wi, chi) in wc_order:
        b = int(B[wi, chi])
        if b == 0:
            continue
        key = (wi // SWG, chi)
        gb0 = int(block_base[wi, chi])
        if prev != key:
            runs.append([chi, gb0, 0])
            prev = key
        runs[-1][2] += b
    for chi, gb0, b in runs:
        off = 0
        while off < b:
            nb = min(MAXB, b - off)
            wins = [int(win_of_block[gb0 + off + j]) for j in range(nb)]
            sched.calls.append((chi, gb0 + off, nb, wins))
            off += nb

    # nonempty windows in flush order = ascending win_last_gb
    sched.nonempty_windows = sorted(
        sched.win_last_gb.keys(), key=lambda wi: sched.win_last_gb[wi]
    )
    return sched


# --------------------------------------------------------------------------
# device program
# --------------------------------------------------------------------------

def build_program(s: Schedule, ntot: int, nlayers: int = 3):
    import concourse.bacc as bacc
    import concourse.bass as bass
    import concourse.mybir as mybir
    from concourse._compat import get_trn_type
    from concourse.library_config import mlp
    from contextlib import ExitStack

    R, CH, NW, NCH, TB = s.R, s.CH, s.NW, s.NCH, s.TB
    NG = -(-TB // SGRP)
    C = len(s.calls)

    nc = bacc.Bacc(get_trn_type() or "TRN2", debug=True)

    xshard = nc.dram_tensor("xshard", [R, D], mybir.dt.float32, kind="ExternalInput")
    idx_in = nc.dram_tensor("idx_img", [128, 8 * TB], mybir.dt.int16, kind="ExternalInput")
    w_in = nc.dram_tensor("w_img", [128, TB], mybir.dt.float32, kind="ExternalInput")
    ldst_in = nc.dram_tensor("ldst_img", [128, TB], mybir.dt.float32, kind="ExternalInput")
    iota_in = nc.dram_tensor("iota", [128, 128], mybir.dt.float32, kind="ExternalInput")
    acc_out = nc.dram_tensor("acc_out", [R, D], mybir.dt.float32, kind="ExternalOutput")

    t_a = nc.dram_tensor("t_a", [ntot, D], mybir.dt.float32, addr_space="Shared")
    t_b = nc.dram_tensor("t_b", [ntot, D], mybir.dt.float32, addr_space="Shared")
    myshard = nc.dram_tensor("myshard", [R, D], mybir.dt.float32)

    tables = [t_a, t_b, t_a]

    def mm_thr(layer, gb):
        return layer * TB + gb + 1

    flush_index = {}
    for li in range(nlayers):
        for k, wi in enumerate(s.nonempty_windows):
            flush_index[(li, wi)] = li * len(s.nonempty_windows) + k + 1
    NF = len(s.nonempty_windows)

    def group_last_gb(g):
        return min((g + 1) * SGRP, TB) - 1

    def shard_ap(t):
        return bass.AP(t, 0, [[D, 128], [WIN * D, NW], [1, D]])

    class W:
        def __init__(self, eng):
            self.eng = eng
            self.best = {}

        def __call__(self, sem, val):
            if val <= 0:
                return
            if self.best.get(id(sem), -1) >= val:
                return
            self.best[id(sem)] = val
            self.eng.wait_ge(sem, val)

    with ExitStack() as st:
        block = st.enter_context(nc.Block())
        sb = lambda name, shape, dt: st.enter_context(nc.sbuf_tensor(name, shape, dt))
        sem = lambda name: st.enter_context(nc.semaphore(name))

        idx_t = sb("idx_t", [128, 8 * TB], mybir.dt.int16)
        w_t = sb("w_t", [128, TB], mybir.dt.float32)
        ldst_t = sb("ldst_t", [128, TB], mybir.dt.float32)
        iota_t = sb("iota_t", [128, 128], mybir.dt.float32)
        acc_t = sb("acc_t", [128, NW, D], mybir.dt.float32)
        h_t = sb("h_t", [128, NW, D], mybir.dt.float32)
        G = [sb(f"G{i}", [128, MAXB, D], mybir.dt.float32) for i in range(4)]
        M = [sb(f"M{i}", [128, MAXB, D], mybir.dt.bfloat16) for i in range(4)]
        S = [sb(f"S{i}", [128, SGRP, 128], mybir.dt.bfloat16) for i in range(2)]
        ps = [st.enter_context(nc.psum_tensor(f"ps{i}", [128, 512], mybir.dt.float32))
              for i in range(8)]

        ld = sem("ld")
        g_sem = sem("g_sem")
        m_done = sem("m_done")
        sb_sem = sem("sb_sem")
        mm_sem = sem("mm_sem")
        f_sem = sem("f_sem")
        cc_sem = sem("cc_sem")
        e_sem = sem("e_sem")
        aa_sem = sem("aa_sem")
        o_sem = sem("o_sem")

        @block.sync
        def _(s_eng):
            wt = W(s_eng)
            s_eng.dma_start(idx_t[:], idx_in[:]).then_inc(ld, 16)
            s_eng.dma_start(w_t[:], w_in[:]).then_inc(ld, 16)
            s_eng.dma_start(ldst_t[:], ldst_in[:]).then_inc(ld, 16)
            s_eng.dma_start(iota_t[:], iota_in[:]).then_inc(ld, 16)
            s_eng.dma_start(acc_t[:], shard_ap(xshard)).then_inc(ld, 16)
            s_eng.dma_start(myshard[:], xshard[:]).then_inc(ld, 16)
            for li in range(nlayers - 1):
                wt(f_sem, (li + 1) * NF)
                wt(cc_sem, li + 1)
                s_eng.dma_start(shard_ap(myshard), h_t[:]).then_inc(e_sem, 16)
            wt(aa_sem, nlayers)
            s_eng.dma_start(shard_ap(acc_out), acc_t[:]).then_inc(o_sem, 16)
            wt(o_sem, 16)

        @block.gpsimd
        def _(g_eng):
            wt = W(g_eng)
            g_eng.load_library(mlp)
            wt(ld, 16 * 6)
            g_eng.collective_compute(
                "AllGather", mybir.AluOpType.bypass,
                replica_groups=[list(range(NCORE))],
                ins=[myshard[:]], outs=[t_a[:]],
            ).then_inc(cc_sem)
            for li in range(nlayers):
                wt(cc_sem, li + 1)
                tbl = tables[li]
                for q, (chi, gb0, nb, wins) in enumerate(s.calls):
                    Q = li * C + q
                    wt(m_done, Q - 3)
                    ni = 128 * nb
                    lo = chi * CH
                    hi = min(lo + CH, ntot)
                    g_eng.dma_gather(
                        G[Q % 4][:, :nb, :], tbl[lo:hi],
                        idx_t[:, 8 * gb0: 8 * (gb0 + nb)],
                        ni, ni, D,
                    ).then_inc(g_sem, 16)
                if li < nlayers - 1:
                    wt(e_sem, 16 * (li + 1))
                    g_eng.collective_compute(
                        "AllGather", mybir.AluOpType.bypass,
                        replica_groups=[list(range(NCORE))],
                        ins=[myshard[:]], outs=[tables[li + 1][:]],
                    ).then_inc(cc_sem)

        @block.vector
        def _(v):
            wt = W(v)
            wt(ld, 16 * 6)
            v.memset(h_t[:], 0.0)
            for li in range(nlayers):
                wt(e_sem, 16 * li)
                for q, (chi, gb0, nb, wins) in enumerate(s.calls):
                    Q = li * C + q
                    for j in range(nb):
                        b = gb0 + j
                        g = b // SGRP
                        gg = li * NG + g
                        if gg - 2 >= 0:
                            pl, pg = divmod(gg - 2, NG)
                            wt(mm_sem, mm_thr(pl, group_last_gb(pg)))
                        v.tensor_tensor(
                            out=S[g % 2][:, b - g * SGRP, :],
                            in0=ldst_t[:, b:b + 1].to_broadcast([128, 128]),
                            in1=iota_t[:],
                            op=mybir.AluOpType.is_equal,
                        ).then_inc(sb_sem)
                    wt(g_sem, 16 * (Q + 1))
                    if Q - 4 >= 0:
                        pl, pqq = divmod(Q - 4, C)
                        pgb0, pnb = s.calls[pqq][1], s.calls[pqq][2]
                        wt(mm_sem, mm_thr(pl, pgb0 + pnb - 1))
                    v.tensor_tensor(
                        out=M[Q % 4][:, :nb, :],
                        in0=G[Q % 4][:, :nb, :],
                        in1=w_t[:, gb0:gb0 + nb].to_broadcast([128, nb, D]),
                        op=mybir.AluOpType.mult,
                    ).then_inc(m_done)
                    for j in range(nb):
                        wi = wins[j]
                        if gb0 + j == s.win_last_gb.get(wi, -1):
                            wt(mm_sem, mm_thr(li, gb0 + j))
                            v.tensor_copy(
                                out=h_t[:, wi, :],
                                in_=ps[wi % 8][:, :D],
                            ).then_inc(f_sem)
                wt(f_sem, (li + 1) * NF)
                v.tensor_add(out=acc_t[:], in0=acc_t[:], in1=h_t[:]).then_inc(aa_sem)

        @block.tensor
        def _(t):
            wt = W(t)
            for li in range(nlayers):
                for q, (chi, gb0, nb, wins) in enumerate(s.calls):
                    Q = li * C + q
                    wt(m_done, Q + 1)
                    for j in range(nb):
                        b = gb0 + j
                        wi = wins[j]
                        g = b // SGRP
                        wt(sb_sem, li * TB + b + 1)
                        first = b == s.win_first_gb.get(wi, -2)
                        last = b == s.win_last_gb.get(wi, -2)
                        if first and wi - SWG >= 0:
                            pfi = flush_index.get((li, wi - SWG))
                            if pfi is not None:
                                wt(f_sem, pfi)
                        t.matmul(
                            ps[wi % 8][:, :D],
                            S[g % 2][:, b - g * SGRP, :],
                            M[Q % 4][:, j, :],
                            start=first, stop=last,
                            skip_group_check=True,
                        ).then_inc(mm_sem)

    nc.compile()
    return nc


# --------------------------------------------------------------------------
# entry points
# --------------------------------------------------------------------------

def _permutation():
    pi = np.empty(N_NODES, np.int64)
    u = np.arange(N_USERS)
    pi[:N_USERS] = 3 * (u // 2) + (u % 2)
    i = np.arange(N_ITEMS)
    pi[N_USERS:] = 3 * i + 2
    return pi


def _kernel_device(user_embedding, item_embedding, edge_weight, edge_index,
                   user_id, item_id):
    global LAST_EXEC_NS
    from concourse import bass_utils

    pi = _permutation()
    src = np.asarray(edge_index[0], np.int64)
    dst = np.asarray(edge_index[1], np.int64)
    w = np.asarray(edge_weight, np.float32)
    x = np.concatenate([
        np.asarray(user_embedding, np.float32),
        np.asarray(item_embedding, np.float32),
    ])
    xp = np.zeros((NTOT, D), np.float32)
    xp[pi] = x

    key = (src.shape[0],)
    if "sched_hash" in _CACHE and _CACHE.get("edges_key") == key:
        pass  # reuse compiled program only if the edge structure is identical
    sched = build_schedule(pi[src], pi[dst], w, NTOT, R, CH)
    shash = hash((sched.TB, repr(sched.calls[:50]), len(sched.calls)))
    if _CACHE.get("sched_hash") != shash:
        _CACHE["nc"] = build_program(sched, NTOT, NUM_LAYERS)
        _CACHE["sched_hash"] = shash
        _CACHE["edges_key"] = key
        try:
            from concourse.timeline_sim import TimelineSim
            _CACHE["est_ns"] = int(TimelineSim(_CACHE["nc"]).simulate())
        except Exception:
            _CACHE["est_ns"] = None
    nc = _CACHE["nc"]
    LAST_EXEC_NS = _CACHE.get("est_ns")

    iota = np.tile(np.arange(128, dtype=np.float32), (128, 1))
    in_maps = []
    for c in range(NCORE):
        in_maps.append({
            "xshard": np.ascontiguousarray(xp[c * R:(c + 1) * R]),
            "idx_img": sched.idx_img[c],
            "w_img": sched.w_img[c],
            "ldst_img": sched.ldst_img[c],
            "iota": iota,
        })
    res = bass_utils.run_bass_kernel_spmd(nc, in_maps, list(range(NCORE)), trace=False)
    acc = np.concatenate([res.results[c]["acc_out"] for c in range(NCORE)], axis=0)

    final = acc[pi] * np.float32(1.0 / (NUM_LAYERS + 1))
    u_embed = np.ascontiguousarray(final[:N_USERS][np.asarray(user_id, np.int64)])
    i_embed = np.ascontiguousarray(final[N_USERS:][np.asarray(item_id, np.int64)])
    if not (np.isfinite(u_embed).all() and np.isfinite(i_embed).all()):
        raise FloatingPointError("non-finite device output")
    return (u_embed, i_embed)


def _kernel_numpy(user_embedding, item_embedding, edge_weight, edge_index,
                  user_id, item_id):
    """Exact host fallback (feature-major reduceat segment sums)."""
    xt = np.empty((D, N_NODES), np.float32)
    xt[:, :N_USERS] = np.asarray(user_embedding, np.float32).T
    xt[:, N_USERS:] = np.asarray(item_embedding, np.float32).T
    src = np.asarray(edge_index[0], np.int64)
    dst = np.asarray(edge_index[1], np.int64)
    w = np.asarray(edge_weight, np.float32)
    user_id = np.asarray(user_id, np.int64)
    item_id = np.asarray(item_id, np.int64)

    order = np.argsort(dst.astype(np.uint32), kind="stable")
    src_s, dst_s, w_s = src[order], dst[order], w[order]
    starts = np.flatnonzero(np.diff(dst_s, prepend=dst_s[0] - 1))
    rows = dst_s[starts]

    def seg(ht):
        m = np.take(ht, src_s, axis=1, mode="clip")
        np.multiply(m, w_s[None, :], out=m)
        out = np.zeros_like(ht)
        out[:, rows] = np.add.reduceat(m, starts, axis=1)
        return out

    h1 = seg(xt)
    h2 = seg(h1)
    h3 = seg(h2)
    qu = user_id
    qi = N_USERS + item_id
    fu = (xt[:, qu] + h1[:, qu] + h2[:, qu] + h3[:, qu]).T * np.float32(0.25)
    fi = (xt[:, qi] + h1[:, qi] + h2[:, qi] + h3[:, qi]).T * np.float32(0.25)
    return (np.ascontiguousarray(fu), np.ascontiguousarray(fi))


def kernel(user_embedding, item_embedding, edge_weight, edge_index,
           user_id, item_id):
    try:
        return _kernel_device(user_embedding, item_embedding, edge_weight,
                              edge_index, user_id, item_id)
    except Exception as e:
        print(f"kernel: device path failed ({type(e).__name__}: {e}); "
              f"using host fallback", file=sys.stderr)
        return _kernel_numpy(user_embedding, item_embedding, edge_weight,
                             edge_index, user_id, item_id)
